# revision 18
# baseline (speedup 1.0000x reference)
"""CrossAttentionBlock Trainium2 kernel — data-parallel over batch across 8 cores.

Full inputs in, full outputs out. Each core handles 2 of the 16 batch
elements; weights are replicated. No collectives.

Math notes (vs the jax reference):
- AdaRMSNorm on x: inv_rms_x cancels through the q cosine-normalization and is
  skipped; the cond-dependent scale s_x is applied to x before the q proj.
- AdaRMSNorm on crossattn_cond: inv_rms_c cancels for k but not v; it is
  folded into v only (together with the 1/16 fp8 weight descale).
- Mask compression: the boolean key mask is applied on the HOST by gathering
  only attendable key rows (max 139 of 256 for this input distribution) and
  padding to LP=192; padded rows get an additive -60 exp bias (exact zero
  after fp8 quantization of E).
- Projections (q/kv/out) and attn@v run as fp8e4m3 DoubleRow matmuls
  (2 contraction rows per cycle). Weights are scaled x16 into fp8; the scale
  cancels in the cosine norms for q/k, and is descaled via the softmax
  denominator (den accumulates 16.0-valued ones) for the out proj.
- Softmax denominators for all 16 heads are accumulated into one [16, 512]
  PSUM tile by per-head fp8 matmuls against a crafted ones-pattern, then
  reciprocal'd and broadcast back per head-pair via an indicator matmul.
- Per head pair (even h0, odd h1) the second key chunk (l 128..192) of both
  heads is packed into one [128, 512] score tile so exp runs 3x per pair
  instead of 4x.
"""

import numpy as np
import ml_dtypes

D_HEAD = 64
EPS = 1e-6
N, H, W, D = 16, 32, 32, 1024
L, DC, CF = 256, 1024, 768
NH = D // D_HEAD  # 16
NCORES = 8
NB = N // NCORES  # 2 batch elements per core
T = H * W  # 1024 tokens per batch element
CH = 512  # token chunk
LP = 192  # compressed+padded key length
MASK_NEG = -60.0
WS = 16.0  # fp8 weight scale

_cached = {}


def _build_nc():
    from contextlib import ExitStack

    import concourse.mybir as mybir
    import concourse.tile as tile
    from concourse import bacc

    f32 = mybir.dt.float32
    f16 = mybir.dt.float16
    f8 = mybir.dt.float8e4
    Exp = mybir.ActivationFunctionType.Exp
    Sqrt = mybir.ActivationFunctionType.Sqrt
    MULT = mybir.AluOpType.mult
    ADD = mybir.AluOpType.add
    DR = mybir.MatmulPerfMode.DoubleRow

    nc = bacc.Bacc(None, target_bir_lowering=False)

    xT = nc.declare_dram_parameter("xT", [NB, D, T], f16, isOutput=False)
    xs_d = nc.declare_dram_parameter("xs", [NB, T, D], f16, isOutput=False)
    ccT = nc.declare_dram_parameter("ccT", [NB, DC, LP], f16, isOutput=False)
    condT = nc.declare_dram_parameter("condT", [CF, NB], f16, isOutput=False)
    mb_d = nc.declare_dram_parameter("mb", [NB, 2, 128], f32, isOutput=False)
    w_nT = nc.declare_dram_parameter("w_nT", [CF, D], f16, isOutput=False)
    w_cT = nc.declare_dram_parameter("w_cT", [CF, DC], f16, isOutput=False)
    wq8_d = nc.declare_dram_parameter("wq8", [D, D], f8, isOutput=False)
    wkv8_d = nc.declare_dram_parameter("wkv8", [DC, 2 * D], f8, isOutput=False)
    wo8_d = nc.declare_dram_parameter("wo8", [D, D], f8, isOutput=False)
    indT = nc.declare_dram_parameter("indT", [D, NH], f16, isOutput=False)
    scl_d = nc.declare_dram_parameter("scl", [NH, 4], f32, isOutput=False)
    dones_d = nc.declare_dram_parameter("dones", [128, 2, NH, NH], f8,
                                        isOutput=False)
    dpa_d = nc.declare_dram_parameter("dpa", [128, 2, NH], f8, isOutput=False)
    dpb_d = nc.declare_dram_parameter("dpb", [128, 2, NH], f8, isOutput=False)
    onesd = nc.declare_dram_parameter("onesd", [128, 1], f16, isOutput=False)
    onesf = nc.declare_dram_parameter("onesf", [1, 1], f32, isOutput=False)
    out = nc.declare_dram_parameter("out", [NB, T, D], f16, isOutput=True)

    P = 128
    NDC = D // P      # 8 contraction chunks of d / d_cross
    NCF = CF // P     # 6 chunks of cond_f
    NJC = D // P      # 8 chunks of head-dim j (2 heads each)
    NKP = NDC // 2    # 4 fp8 DoubleRow contraction pairs

    def mm(ps_, lhsT, rhs, start, stop):
        nc.tensor.matmul(ps_, lhsT, rhs, start=start, stop=stop)

    def mm8(ps_, lhsT, rhs, start, stop):
        nc.tensor.matmul(ps_, lhsT, rhs, start=start, stop=stop, perf_mode=DR)

    def act_raw(out, in_, func, bias, scale):
        # InstActivation via the public emission API; Rsqrt/Reciprocal are
        # accurate to ~3e-4 here (validated on hw), well inside the 2e-2 gate.
        eng = nc.scalar
        inputs = [eng.lower_ap(in_)]
        for arg in (bias, scale, 0.0):
            if isinstance(arg, float):
                inputs.append(mybir.ImmediateValue(dtype=mybir.dt.float32,
                                                   value=arg))
            else:
                inputs.append(eng.lower_ap(arg))
        return eng.add_instruction(mybir.InstActivation(
            name=nc.get_next_instruction_name(), func=func, ins=inputs,
            outs=[eng.lower_ap(out)]))

    Rsqrt = mybir.ActivationFunctionType.Rsqrt
    Recip = mybir.ActivationFunctionType.Reciprocal

    with tile.TileContext(nc) as tc, ExitStack() as ctx:
        ctx.enter_context(nc.allow_low_precision(
            reason="fp8 matmuls and f16 staging validated against 2e-2 gate"))
        const = ctx.enter_context(tc.tile_pool(name="const", bufs=1))
        acts = ctx.enter_context(tc.tile_pool(name="acts", bufs=1))
        small = ctx.enter_context(tc.tile_pool(name="small", bufs=2))
        ps = ctx.enter_context(tc.tile_pool(name="ps", bufs=1, space="PSUM"))
        dscr = ctx.enter_context(tc.tile_pool(name="dscr", bufs=2, space="DRAM"))

        # ---- constants ----
        ones = const.tile([P, 1], f16)
        nc.sync.dma_start(out=ones, in_=onesd[:])
        onef = const.tile([1, 1], f32)
        nc.sync.dma_start(out=onef, in_=onesf[:])
        eps_t = const.tile([P, 1], f32)
        nc.vector.memset(eps_t, EPS)
        eps256 = const.tile([P, 1], f32)
        nc.vector.memset(eps256, EPS * WS * WS)
        indT_sb = const.tile([P, NJC, NH], f16)
        nc.sync.dma_start(out=indT_sb, in_=indT.rearrange("(jc p) h -> p jc h", p=P))
        scl = const.tile([NH, 4], f32)
        nc.sync.dma_start(out=scl, in_=scl_d[:])
        dones = const.tile([128, 2, NH, NH], f8)
        nc.sync.dma_start(out=dones, in_=dones_d[:])
        dpa = const.tile([128, 2, NH], f8)
        nc.sync.dma_start(out=dpa, in_=dpa_d[:])
        dpb = const.tile([128, 2, NH], f8)
        nc.sync.dma_start(out=dpb, in_=dpb_d[:])
        mb_sb = const.tile([P, 2, NB], f32)
        cond_sb = const.tile([P, NCF, NB], f16)
        for b in range(NB):
            nc.sync.dma_start(out=mb_sb[:, :, b],
                              in_=mb_d[b].rearrange("s p -> p s"))
            nc.sync.dma_start(out=cond_sb[:, :, b],
                              in_=condT[:, b].rearrange("(c p) -> p c", p=P))
        s_x = const.tile([P, NDC, NB], f32)
        s_c = const.tile([P, NDC, NB], f32)
        # gam columns: 0 = l 0:128 (rows 0:128), 1 = l 128:192 (rows 0:64),
        # 2 = l 128:192 (rows 64:128); all include the 1/16 wkv descale.
        gam = const.tile([P, 3, NB], f32)

        # fp8 weights
        wq8 = const.tile([P, NDC, D], f8)
        nc.sync.dma_start(out=wq8, in_=wq8_d.rearrange("(c p) j -> p c j", p=P))
        wkv8 = const.tile([P, NDC, 2 * D], f8)
        nc.sync.dma_start(out=wkv8, in_=wkv8_d.rearrange("(c p) j -> p c j", p=P))
        wo8 = const.tile([P, NJC, D], f8)
        nc.sync.dma_start(out=wo8, in_=wo8_d.rearrange("(c p) j -> p c j", p=P))

        # ---- stage A: s_x = cond @ w_norm.T + 1, s_c = cond @ w_cnorm.T + 1 ----
        with tc.tile_pool(name="pnorm", bufs=1) as pnorm:
            for wdram, dst in ((w_cT, s_c), (w_nT, s_x)):
                w_sb = pnorm.tile([P, NCF, D], f16, tag="wnorm", bufs=2)
                nc.sync.dma_start(out=w_sb,
                                  in_=wdram.rearrange("(c p) j -> p c j", p=P))
                sps = ps.tile([P, NDC, NB], f32, tag="stat", bufs=1)
                for jc in range(NDC):
                    for c in range(NCF):
                        mm(sps[:, jc, :], w_sb[:, c, jc * P:(jc + 1) * P],
                           cond_sb[:, c, :], start=(c == 0), stop=(c == NCF - 1))
                nc.vector.tensor_scalar_add(dst[:], sps[:], 1.0)

        # ---- stage B: kT8 (cosine-normalized fp8) and vtE/vtO per batch ----
        kt8_sb = []   # [128(j), NJC, LP] fp8
        ktz_sb = []   # [128(j), NJC, 128] fp8 — c1 keys split per head half,
                      # zero elsewhere, so one M=128 matmul scores both heads
        vtE_sb = []   # [128(l), 2(pair), NH, 64] fp8 — even heads, odd cols 0
        vtO_sb = []   # same — odd heads, even cols 0
        for b in range(NB):
            kt8_sb.append(acts.tile([P, NJC, LP], f8, name=f"kt8{b}"))
            ktz_sb.append(acts.tile([P, NJC, P], f8, name=f"ktz{b}"))
            vtE_sb.append(acts.tile([P, 2, NH, D_HEAD], f8, name=f"vtE{b}"))
            vtO_sb.append(acts.tile([P, 2, NH, D_HEAD], f8, name=f"vtO{b}"))
        with tc.tile_pool(name="pkv", bufs=1) as pkv:
            def b_prep(b):
                """cc load, gamma stats, fp8 quantize of scaled cc."""
                nc.vector.memset(vtE_sb[b][:], 0.0)
                nc.vector.memset(vtO_sb[b][:], 0.0)
                nc.vector.memset(ktz_sb[b][:], 0.0)
                cc = pkv.tile([P, NDC, LP], f16, tag="cc", bufs=2)
                nc.sync.dma_start(out=cc,
                                  in_=ccT[b].rearrange("(c p) l -> p c l", p=P))
                ccsq = pkv.tile([P, NDC, LP], f16, tag="ccsq", bufs=1)
                nc.gpsimd.tensor_mul(ccsq[:], cc[:], cc[:])
                msq = ps.tile([1, LP], f32, tag="stat", bufs=1)
                for c in range(NDC):
                    mm(msq, ones, ccsq[:, c, :], start=(c == 0), stop=(c == NDC - 1))
                gr = small.tile([1, LP], f32, tag="gamr")
                act_raw(gr, msq, Rsqrt, eps256[:1], WS * WS / DC)
                grpad = small.tile([1, P], f32, tag="grpad")
                nc.vector.memset(grpad[:, 0:D_HEAD], 0.0)
                nc.vector.tensor_copy(out=grpad[:, D_HEAD:P], in_=gr[:, P:LP])
                gps = ps.tile([P, 3], f32, tag="stat", bufs=1)
                nc.tensor.matmul(gps[:, 0:1], gr[0:1, 0:P], onef[:],
                                 start=True, stop=True)
                nc.tensor.matmul(gps[0:D_HEAD, 1:2], gr[0:1, P:LP], onef[:],
                                 start=True, stop=True)
                nc.tensor.matmul(gps[:, 2:3], grpad[0:1, :], onef[:],
                                 start=True, stop=True)
                nc.scalar.copy(out=gam[:, :, b], in_=gps)
                cc8 = pkv.tile([P, NDC, 320], f8, tag="cc8", bufs=2)
                nc.vector.memset(cc8[:, :, LP:256], 0.0)
                for c in range(NDC):
                    nc.vector.tensor_scalar_mul(cc8[:, c, 0:LP], cc[:, c, :],
                                                s_c[:, c, b:b + 1])
                    nc.vector.tensor_scalar_mul(cc8[:, c, 256:320],
                                                cc[:, c, P:LP],
                                                s_c[:, c, b:b + 1])
                return cc8

            def b_kproj(b, cc8):
                """k projection + norm stats."""
                ktf = pkv.tile([P, NJC, LP], f16, tag="ktf", bufs=2)
                ksq = pkv.tile([P, NJC, LP], f16, tag="ksq", bufs=1)
                kss = ps.tile([NH, LP], f32, tag="stat", bufs=1)
                for jc in range(NJC):
                    kps = ps.tile([P, CH], f32, tag="qps", bufs=2)
                    for i in range(NKP):
                        mm8(kps[:, 0:LP],
                            wkv8[:, 2 * i:2 * i + 2, jc * P:(jc + 1) * P],
                            cc8[:, 2 * i:2 * i + 2, 0:LP],
                            start=(i == 0), stop=(i == NKP - 1))
                    nc.scalar.copy(out=ktf[:, jc, :], in_=kps[:, 0:LP])
                    nc.gpsimd.tensor_mul(ksq[:, jc, :], ktf[:, jc, :],
                                         ktf[:, jc, :])
                    mm(kss, indT_sb[:, jc, :], ksq[:, jc, :],
                       start=(jc == 0), stop=(jc == NJC - 1))
                gkT = small.tile([NH, LP], f16, tag="gkT")
                act_raw(gkT, kss, Rsqrt, scl[:, 3:4], scl[:, 2:3])
                gkd = dscr.tile([NH, LP], f16, tag="gkd", bufs=2)
                nc.sync.dma_start(out=gkd, in_=gkT)
                gkbs = []
                for jc in range(NJC):
                    gkb = pkv.tile([P, LP], f16, tag="gkb", bufs=2 * NJC)
                    nc.sync.dma_start(
                        out=gkb[0:D_HEAD, :],
                        in_=gkd[2 * jc:2 * jc + 1, :].to_broadcast((D_HEAD, LP)))
                    nc.sync.dma_start(
                        out=gkb[D_HEAD:P, :],
                        in_=gkd[2 * jc + 1:2 * jc + 2, :].to_broadcast((D_HEAD, LP)))
                    gkbs.append(gkb)
                return ktf, gkbs

            def b_vproj(b, cc8):
                """v projection, gamma scale, parity split into vtE/vtO."""
                vtE, vtO = vtE_sb[b], vtO_sb[b]
                for vjc in range(2):
                    vps = ps.tile([P, CH], f32, tag="att", bufs=4)
                    for i in range(NKP):
                        mm8(vps, cc8[:, 2 * i:2 * i + 2, 0:P],
                            wkv8[:, 2 * i:2 * i + 2, D + vjc * CH:D + (vjc + 1) * CH],
                            start=(i == 0), stop=(i == NKP - 1))
                    he = vps.rearrange("p (h e) -> p h e", e=D_HEAD)
                    nc.vector.tensor_scalar_mul(
                        vtE[:, 0, vjc * 8:(vjc + 1) * 8:2, :],
                        he[:, 0::2, :], gam[:, 0, b:b + 1])
                    nc.vector.tensor_scalar_mul(
                        vtO[:, 1, vjc * 8 + 1:(vjc + 1) * 8:2, :],
                        he[:, 1::2, :], gam[:, 0, b:b + 1])
                    vpsA = ps.tile([P, CH], f32, tag="att", bufs=4)
                    for i in range(NKP):
                        mm8(vpsA[0:D_HEAD, :], cc8[:, 2 * i:2 * i + 2, P:LP],
                            wkv8[:, 2 * i:2 * i + 2, D + vjc * CH:D + (vjc + 1) * CH],
                            start=(i == 0), stop=(i == NKP - 1))
                    heA = vpsA.rearrange("p (h e) -> p h e", e=D_HEAD)
                    nc.vector.tensor_scalar_mul(
                        vtE[0:D_HEAD, 1, vjc * 8:(vjc + 1) * 8:2, :],
                        heA[0:D_HEAD, 0::2, :], gam[0:D_HEAD, 1, b:b + 1])
                    vpsB = ps.tile([P, CH], f32, tag="att", bufs=4)
                    for i in range(NKP):
                        mm8(vpsB, cc8[:, 2 * i:2 * i + 2, 192:320],
                            wkv8[:, 2 * i:2 * i + 2, D + vjc * CH:D + (vjc + 1) * CH],
                            start=(i == 0), stop=(i == NKP - 1))
                    heB = vpsB.rearrange("p (h e) -> p h e", e=D_HEAD)
                    nc.vector.tensor_scalar_mul(
                        vtO[D_HEAD:P, 0, vjc * 8 + 1:(vjc + 1) * 8:2, :],
                        heB[D_HEAD:P, 1::2, :], gam[D_HEAD:P, 2, b:b + 1])

            def b_knorm(b, ktf, gkbs):
                kt8, ktz = kt8_sb[b], ktz_sb[b]
                for jc in range(NJC):
                    nc.vector.tensor_tensor(kt8[:, jc, 0:P], ktf[:, jc, 0:P],
                                            gkbs[jc][:, 0:P], MULT)
                    nc.vector.tensor_tensor(
                        ktz[0:D_HEAD, jc, 0:D_HEAD],
                        ktf[0:D_HEAD, jc, P:LP],
                        gkbs[jc][0:D_HEAD, P:LP], MULT)
                    nc.vector.tensor_tensor(
                        ktz[D_HEAD:P, jc, D_HEAD:P],
                        ktf[D_HEAD:P, jc, P:LP],
                        gkbs[jc][D_HEAD:P, P:LP], MULT)

            stageB = {"prep": b_prep, "kproj": b_kproj, "vproj": b_vproj,
                      "knorm": b_knorm}

            # ---- stages C/D/E: stream 512-token chunks ----
            pw2 = pkv
            NCH = NB * (T // CH)

            def phase1(chunk):
                """xt load + xq8 quantize only."""
                b, th = chunk // (T // CH), chunk % (T // CH)
                tsl = slice(th * CH, (th + 1) * CH)
                xt = pw2.tile([P, NDC, CH], f16, tag="xt", bufs=2)
                nc.sync.dma_start(
                    out=xt, in_=xT[b].rearrange("(c p) t -> p c t", p=P)[:, :, tsl])
                xq8 = pw2.tile([P, NDC, CH], f8, tag="xq8", bufs=2)
                for c in range(NDC):
                    nc.vector.tensor_scalar_mul(xq8[:, c, :], xt[:, c, :],
                                                s_x[:, c, b:b + 1])
                return {"b": b, "th": th, "xq8": xq8}

            def qproj_work(st1):
                """Returns (8 per-jc filler closures, finalize) for the q proj
                of the chunk described by st1."""
                b, xq8 = st1["b"], st1["xq8"]
                qf = pw2.tile([P, NJC, CH], f16, tag="qf", bufs=2)
                qsq = pw2.tile([P, NJC, CH], f16, tag="qsq", bufs=1)
                qss = ps.tile([NH, CH], f32, tag="stat", bufs=1)

                def filler(jc):
                    qps = ps.tile([P, CH], f32, tag="qps", bufs=2)
                    for i in range(NKP):
                        mm8(qps, wq8[:, 2 * i:2 * i + 2, jc * P:(jc + 1) * P],
                            xq8[:, 2 * i:2 * i + 2, :],
                            start=(i == 0), stop=(i == NKP - 1))
                    nc.scalar.copy(out=qf[:, jc, :], in_=qps)
                    nc.gpsimd.tensor_mul(qsq[:, jc, :], qf[:, jc, :],
                                         qf[:, jc, :])
                    mm(qss, indT_sb[:, jc, :], qsq[:, jc, :],
                       start=(jc == 0), stop=(jc == NJC - 1))

                def finalize():
                    gqT = small.tile([NH, CH], f16, tag="gqT")
                    act_raw(gqT, qss, Rsqrt, scl[:, 1:2], scl[:, 0:1])
                    gqd = dscr.tile([NH, CH], f16, tag="gqd", bufs=2)
                    nc.sync.dma_start(out=gqd, in_=gqT)
                    gqbs = []
                    for m in range(NJC):
                        gqb = pw2.tile([P, CH], f16, tag="gqb", bufs=12)
                        nc.sync.dma_start(
                            out=gqb[0:D_HEAD, :],
                            in_=gqd[2 * m:2 * m + 1, :].to_broadcast((D_HEAD, CH)))
                        nc.sync.dma_start(
                            out=gqb[D_HEAD:P, :],
                            in_=gqd[2 * m + 1:2 * m + 2, :].to_broadcast(
                                (D_HEAD, CH)))
                        gqbs.append(gqb)
                    return {"b": st1["b"], "th": st1["th"], "qf": qf,
                            "gqbs": gqbs}

                return [lambda jc=jc: filler(jc) for jc in range(NJC)], finalize

            def phase2(st, nxt1=None):
                """Attention pair loop; embeds the next chunk's q proj as
                PE bubble-filler, plus its xq8 quantize ops spread early."""
                b, qf, gqbs = st["b"], st["qf"], st["gqbs"]
                kt8, vtE, vtO = kt8_sb[b], vtE_sb[b], vtO_sb[b]
                ktz = ktz_sb[b]
                fillers, finalize = (qproj_work(nxt1) if nxt1 is not None
                                     else (None, None))
                q8 = pw2.tile([P, NJC, CH], f8, tag="q8", bufs=2)
                o8 = pw2.tile([P, NJC, CH], f8, tag="o8", bufs=2)
                oaps = [None] * NJC
                dbps = [None] * NJC

                def stt_div(k):
                    nc.vector.scalar_tensor_tensor(
                        o8[:, k, :], oaps[k], 0.0, dbps[k],
                        mybir.AluOpType.bypass, MULT)
                for m in range(NJC):
                    h0, h1 = 2 * m, 2 * m + 1
                    if m >= 1:
                        stt_div(m - 1)
                    nc.vector.tensor_tensor(q8[:, m, :], qf[:, m, :], gqbs[m],
                                             MULT)
                    E2 = pw2.tile([P, 3, CH], f8, tag="E2", bufs=3)
                    scpA = ps.tile([P, CH], f32, tag="att", bufs=4)
                    mm(scpA, kt8[0:D_HEAD, m, 0:P], q8[0:D_HEAD, m, :],
                       start=True, stop=True)
                    nc.scalar.activation(out=E2[:, 0, :], in_=scpA, func=Exp,
                                         bias=mb_sb[:, 0, b:b + 1], scale=1.0)
                    scpB = ps.tile([P, CH], f32, tag="att", bufs=4)
                    mm(scpB, ktz[:, m, :], q8[:, m, :], start=True, stop=True)
                    nc.scalar.activation(out=E2[:, 1, :], in_=scpB, func=Exp,
                                         bias=mb_sb[:, 1, b:b + 1], scale=1.0)
                    scpC = ps.tile([P, CH], f32, tag="att", bufs=4)
                    mm(scpC, kt8[D_HEAD:P, m, 0:P], q8[D_HEAD:P, m, :],
                       start=True, stop=True)
                    nc.scalar.activation(out=E2[:, 2, :], in_=scpC, func=Exp,
                                         bias=mb_sb[:, 0, b:b + 1], scale=1.0)
                    if fillers is not None and 3 <= m:
                        fillers[m - 3]()
                    den_m = ps.tile([2, CH], f32, tag="den", bufs=1)
                    mm8(den_m, dpa[:, :, h0:h0 + 2], E2[:, 0:2, :],
                        start=True, stop=False)
                    mm8(den_m, dpb[:, :, h0:h0 + 2], E2[:, 1:3, :],
                        start=False, stop=True)
                    oap = ps.tile([P, CH], f32, tag="att", bufs=4)
                    mm8(oap, vtE[:, :, h0:h0 + 2, :], E2[:, 0:2, :],
                        start=True, stop=False)
                    mm8(oap, vtO[:, :, h0:h0 + 2, :], E2[:, 1:3, :],
                        start=False, stop=True)
                    oaps[m] = oap
                    den_f = small.tile([2, CH], f32, tag="denf")
                    nc.scalar.copy(out=den_f, in_=den_m)
                    den_r = small.tile([2, CH], f32, tag="denr")
                    nc.vector.reciprocal_approx_fast(out=den_r, in_=den_f)
                    dend = dscr.tile([2, CH], f32, tag="dend", bufs=4)
                    nc.sync.dma_start(out=dend, in_=den_r)
                    dbp = pw2.tile([P, CH], f32, tag="dbpb", bufs=NJC)
                    nc.sync.dma_start(
                        out=dbp[0:D_HEAD, :],
                        in_=dend[0:1, :].to_broadcast((D_HEAD, CH)))
                    nc.sync.dma_start(
                        out=dbp[D_HEAD:P, :],
                        in_=dend[1:2, :].to_broadcast((D_HEAD, CH)))
                    dbps[m] = dbp
                stt_div(NJC - 1)
                nxt = None
                if fillers is not None:
                    for jc in range(NJC - 3, NJC):
                        fillers[jc]()
                    nxt = finalize()
                st.update({"o8": o8})
                return st, nxt

            def phase3(st):
                """Divide by denominators, out projection, skip add, store."""
                b, th, o8 = st["b"], st["th"], st["o8"]
                for t4 in range(CH // P):
                    trow = th * CH + t4 * P
                    xs = small.tile([P, D], f16, tag="xs")
                    nc.sync.dma_start(out=xs, in_=xs_d[b, trow:trow + P, :])
                    os_ = small.tile([P, D], f16, tag="os")
                    for d2 in range(2):
                        ops = ps.tile([P, CH], f32, tag="qps", bufs=2)
                        for i in range(NKP):
                            mm8(ops, o8[:, 2 * i:2 * i + 2, t4 * P:(t4 + 1) * P],
                                wo8[:, 2 * i:2 * i + 2, d2 * CH:(d2 + 1) * CH],
                                start=(i == 0), stop=(i == NKP - 1))
                        nc.vector.tensor_tensor(os_[:, d2 * CH:(d2 + 1) * CH], ops,
                                                xs[:, d2 * CH:(d2 + 1) * CH], ADD)
                    nc.sync.dma_start(out=out[b, trow:trow + P, :], in_=os_)

            # schedule: stage B b=0 first, chunk0 qproj standalone, then the
            # pipelined megas with stage B b=1 slotted between early chunks.
            cc80 = stageB["prep"](0)
            st1_0 = phase1(0)
            ks0 = stageB["kproj"](0, cc80)
            stageB["vproj"](0, cc80)
            stageB["knorm"](0, *ks0)
            f0, fin0 = qproj_work(st1_0)
            for f in f0:
                f()
            st0 = fin0()
            cc81 = stageB["prep"](1)
            st1_1 = phase1(1)
            st2, stA = phase2(st0, st1_1)
            ks1 = stageB["kproj"](1, cc81)
            st1_2 = phase1(2)
            phase3(st2)
            stageB["vproj"](1, cc81)
            stageB["knorm"](1, *ks1)
            st2, stB = phase2(stA, st1_2)
            st1_3 = phase1(3)
            phase3(st2)
            st2, stC = phase2(stB, st1_3)
            phase3(st2)
            st2, _ = phase2(stC, None)
            phase3(st2)

    nc.compile()
    return nc


def _prep_inputs(x, cond, crossattn_cond, crossattn_mask, w_norm, w_q, w_cnorm,
                 w_kv, qk_scale, w_o):
    """Shard + lay out the full inputs into 8 per-core input maps."""
    f = np.float32
    h = np.float16
    e4 = ml_dtypes.float8_e4m3

    # den ones pattern: col h nonzero = WS; even h: (slot0 all, slot1 rows 0:64);
    # odd h: (slot0 rows 64:128, slot1 all)
    dones = np.zeros((128, 2, NH, NH), e4)
    for hh in range(NH):
        if hh % 2 == 0:
            dones[:, 0, hh, hh] = e4(WS)
            dones[0:64, 1, hh, hh] = e4(WS)
        else:
            dones[64:128, 0, hh, hh] = e4(WS)
            dones[:, 1, hh, hh] = e4(WS)

    dpa = np.zeros((128, 2, NH), e4)
    dpb = np.zeros((128, 2, NH), e4)
    for hh in range(NH):
        if hh % 2 == 0:
            dpa[:, 0, hh] = e4(WS)
            dpa[0:64, 1, hh] = e4(WS)
        else:
            dpb[64:128, 0, hh] = e4(WS)
            dpb[:, 1, hh] = e4(WS)
    shared = {
        "dpa": dpa,
        "dpb": dpb,
        "w_nT": np.ascontiguousarray(w_norm.T).astype(h),
        "w_cT": np.ascontiguousarray(w_cnorm.T).astype(h),
        "wq8": (np.ascontiguousarray(w_q.T) * WS).astype(e4),
        "wkv8": (np.ascontiguousarray(w_kv.T) * WS).astype(e4),
        "wo8": (np.ascontiguousarray(w_o.T) * WS).astype(e4),
        "indT": np.kron(np.eye(NH, dtype=h), np.ones((D_HEAD, 1), dtype=h)),
        "scl": np.stack([64.0 / qk_scale.astype(f),
                         EPS * 64.0 / qk_scale.astype(f),
                         1.0 / qk_scale.astype(f),
                         EPS / qk_scale.astype(f)], axis=1).astype(f),
        "dones": dones,
        "onesd": np.ones((128, 1), dtype=h),
        "onesf": np.ones((1, 1), dtype=f),
    }
    in_maps = []
    for c in range(NCORES):
        s = slice(c * NB, (c + 1) * NB)
        xc = np.ascontiguousarray(x[s], dtype=f).reshape(NB, T, D)
        ccg = np.zeros((NB, DC, LP), h)
        mb = np.full((NB, 2, 128), f(MASK_NEG), f)
        for b in range(NB):
            idx = np.nonzero(crossattn_mask[s][b])[0]
            cnt = len(idx)
            assert cnt <= LP, f"mask count {cnt} exceeds LP={LP}"
            ccg[b, :, :cnt] = crossattn_cond[s][b][idx].T.astype(h)
            mb[b, 0, :min(cnt, 128)] = 0.0
            if cnt > 128:
                r = cnt - 128
                mb[b, 1, 0:r] = 0.0
                mb[b, 1, 64:64 + r] = 0.0
        m = {
            "xs": xc.astype(h),
            "xT": np.ascontiguousarray(xc.transpose(0, 2, 1)).astype(h),
            "ccT": ccg,
            "condT": np.ascontiguousarray(cond[s].T, dtype=f).astype(h),
            "mb": mb,
        }
        m.update(shared)
        in_maps.append(m)
    return in_maps


def _run(inputs, trace=False):
    from concourse.bass_utils import run_bass_kernel_spmd

    if "nc" not in _cached:
        _cached["nc"] = _build_nc()
    nc = _cached["nc"]
    in_maps = _prep_inputs(**inputs)
    res = run_bass_kernel_spmd(nc, in_maps, core_ids=list(range(NCORES)),
                               trace=trace)
    outs = np.concatenate([r["out"] for r in res.results], axis=0)
    return outs.reshape(N, H, W, D).astype(np.float32), res


def kernel(**inputs):
    out, _ = _run(inputs, trace=False)
    return out


# revision 20
# speedup vs baseline: 1.0966x; 1.0966x over previous
"""CrossAttentionBlock Trainium2 kernel — data-parallel over batch across 8 cores.

Full inputs in, full outputs out. Each core handles 2 of the 16 batch
elements; weights are replicated. No collectives.

Math notes (vs the jax reference):
- AdaRMSNorm on x: inv_rms_x cancels through the q cosine-normalization and is
  skipped; the cond-dependent scale s_x is applied to x before the q proj.
- AdaRMSNorm on crossattn_cond: inv_rms_c cancels for k but not v; it is
  folded into v only (together with the 1/16 fp8 weight descale).
- Mask compression: the boolean key mask is applied on the HOST by gathering
  only attendable key rows (max 139 of 256 for this input distribution) and
  padding to LP=192; padded rows get an additive -60 exp bias (exact zero
  after fp8 quantization of E).
- Projections (q/kv/out) and attn@v run as fp8e4m3 DoubleRow matmuls
  (2 contraction rows per cycle). Weights are scaled x16 into fp8; the scale
  cancels in the cosine norms for q/k, and is descaled via the softmax
  denominator (den accumulates 16.0-valued ones) for the out proj.
- Softmax denominators for all 16 heads are accumulated into one [16, 512]
  PSUM tile by per-head fp8 matmuls against a crafted ones-pattern, then
  reciprocal'd and broadcast back per head-pair via an indicator matmul.
- Per head pair (even h0, odd h1) the second key chunk (l 128..192) of both
  heads is packed into one [128, 512] score tile so exp runs 3x per pair
  instead of 4x.
"""

import numpy as np
import ml_dtypes

D_HEAD = 64
EPS = 1e-6
N, H, W, D = 16, 32, 32, 1024
L, DC, CF = 256, 1024, 768
NH = D // D_HEAD  # 16
NCORES = 8
NB = N // NCORES  # 2 batch elements per core
T = H * W  # 1024 tokens per batch element
CH = 512  # token chunk
LP = 192  # compressed+padded key length
MASK_NEG = -60.0
WS = 16.0  # fp8 weight scale

_cached = {}


def _build_nc():
    from contextlib import ExitStack

    import concourse.mybir as mybir
    import concourse.tile as tile
    from concourse import bacc

    f32 = mybir.dt.float32
    f16 = mybir.dt.float16
    f8 = mybir.dt.float8e4
    Exp = mybir.ActivationFunctionType.Exp
    Sqrt = mybir.ActivationFunctionType.Sqrt
    MULT = mybir.AluOpType.mult
    ADD = mybir.AluOpType.add
    DR = mybir.MatmulPerfMode.DoubleRow

    nc = bacc.Bacc(None, target_bir_lowering=False)

    xT = nc.declare_dram_parameter("xT", [NB, D, T], f16, isOutput=False)
    xs_d = nc.declare_dram_parameter("xs", [NB, T, D], f16, isOutput=False)
    ccT = nc.declare_dram_parameter("ccT", [NB, DC, LP], f16, isOutput=False)
    condT = nc.declare_dram_parameter("condT", [CF, NB], f16, isOutput=False)
    mb_d = nc.declare_dram_parameter("mb", [NB, 2, 128], f32, isOutput=False)
    w_nT = nc.declare_dram_parameter("w_nT", [CF, D], f16, isOutput=False)
    w_cT = nc.declare_dram_parameter("w_cT", [CF, DC], f16, isOutput=False)
    wq8_d = nc.declare_dram_parameter("wq8", [D, D], f8, isOutput=False)
    wkv8_d = nc.declare_dram_parameter("wkv8", [DC, 2 * D], f8, isOutput=False)
    wo8_d = nc.declare_dram_parameter("wo8", [D, D], f8, isOutput=False)
    indT = nc.declare_dram_parameter("indT", [D, NH], f16, isOutput=False)
    scl_d = nc.declare_dram_parameter("scl", [NH, 4], f32, isOutput=False)
    dpa_d = nc.declare_dram_parameter("dpa", [128, 2, NH], f8, isOutput=False)
    dpb_d = nc.declare_dram_parameter("dpb", [128, 2, NH], f8, isOutput=False)
    onesd = nc.declare_dram_parameter("onesd", [128, 1], f16, isOutput=False)
    onesf = nc.declare_dram_parameter("onesf", [1, 1], f32, isOutput=False)
    out = nc.declare_dram_parameter("out", [NB, T, D], f16, isOutput=True)

    P = 128
    NDC = D // P      # 8 contraction chunks of d / d_cross
    NCF = CF // P     # 6 chunks of cond_f
    NJC = D // P      # 8 chunks of head-dim j (2 heads each)
    NKP = NDC // 2    # 4 fp8 DoubleRow contraction pairs

    def mm(ps_, lhsT, rhs, start, stop):
        nc.tensor.matmul(ps_, lhsT, rhs, start=start, stop=stop)

    def mm8(ps_, lhsT, rhs, start, stop):
        nc.tensor.matmul(ps_, lhsT, rhs, start=start, stop=stop, perf_mode=DR)

    def act_raw(out, in_, func, bias, scale):
        # InstActivation via the public emission API; Rsqrt/Reciprocal are
        # accurate to ~3e-4 here (validated on hw), well inside the 2e-2 gate.
        eng = nc.scalar
        inputs = [eng.lower_ap(in_)]
        for arg in (bias, scale, 0.0):
            if isinstance(arg, float):
                inputs.append(mybir.ImmediateValue(dtype=mybir.dt.float32,
                                                   value=arg))
            else:
                inputs.append(eng.lower_ap(arg))
        return eng.add_instruction(mybir.InstActivation(
            name=nc.get_next_instruction_name(), func=func, ins=inputs,
            outs=[eng.lower_ap(out)]))

    Rsqrt = mybir.ActivationFunctionType.Rsqrt
    Recip = mybir.ActivationFunctionType.Reciprocal

    with tile.TileContext(nc) as tc, ExitStack() as ctx:
        ctx.enter_context(nc.allow_low_precision(
            reason="fp8 matmuls and f16 staging validated against 2e-2 gate"))
        const = ctx.enter_context(tc.tile_pool(name="const", bufs=1))
        acts = ctx.enter_context(tc.tile_pool(name="acts", bufs=1))
        small = ctx.enter_context(tc.tile_pool(name="small", bufs=2))
        ps = ctx.enter_context(tc.tile_pool(name="ps", bufs=1, space="PSUM"))
        dscr = ctx.enter_context(tc.tile_pool(name="dscr", bufs=2, space="DRAM"))

        # ---- constants ----
        ones = const.tile([P, 1], f16)
        nc.sync.dma_start(out=ones, in_=onesd[:])
        onef = const.tile([1, 1], f32)
        nc.sync.dma_start(out=onef, in_=onesf[:])
        eps_t = const.tile([P, 1], f32)
        nc.vector.memset(eps_t, EPS)
        eps256 = const.tile([P, 1], f32)
        nc.vector.memset(eps256, EPS * WS * WS)
        indT_sb = const.tile([P, NJC, NH], f16)
        nc.sync.dma_start(out=indT_sb, in_=indT.rearrange("(jc p) h -> p jc h", p=P))
        scl = const.tile([NH, 4], f32)
        nc.sync.dma_start(out=scl, in_=scl_d[:])
        dpa = const.tile([128, 2, NH], f8)
        nc.sync.dma_start(out=dpa, in_=dpa_d[:])
        dpb = const.tile([128, 2, NH], f8)
        nc.sync.dma_start(out=dpb, in_=dpb_d[:])
        mb_sb = const.tile([P, 2, NB], f32)
        cond_sb = const.tile([P, NCF, NB], f16)
        for b in range(NB):
            nc.sync.dma_start(out=mb_sb[:, :, b],
                              in_=mb_d[b].rearrange("s p -> p s"))
            nc.sync.dma_start(out=cond_sb[:, :, b],
                              in_=condT[:, b].rearrange("(c p) -> p c", p=P))
        s_x = const.tile([P, NDC, NB], f32)
        s_c = const.tile([P, NDC, NB], f32)
        # gam columns: 0 = l 0:128 (rows 0:128), 1 = l 128:192 (rows 0:64),
        # 2 = l 128:192 (rows 64:128); all include the 1/16 wkv descale.
        gam = const.tile([P, 3, NB], f32)

        # fp8 weights
        wq8 = const.tile([P, NDC, D], f8)
        nc.sync.dma_start(out=wq8, in_=wq8_d.rearrange("(c p) j -> p c j", p=P))
        wkv8 = const.tile([P, NDC, 2 * D], f8)
        nc.sync.dma_start(out=wkv8, in_=wkv8_d.rearrange("(c p) j -> p c j", p=P))
        wo8 = const.tile([P, NJC, D], f8)
        nc.sync.dma_start(out=wo8, in_=wo8_d.rearrange("(c p) j -> p c j", p=P))

        # ---- stage A: s_x = cond @ w_norm.T + 1, s_c = cond @ w_cnorm.T + 1 ----
        with tc.tile_pool(name="pnorm", bufs=1) as pnorm:
            for wdram, dst in ((w_cT, s_c), (w_nT, s_x)):
                w_sb = pnorm.tile([P, NCF, D], f16, tag="wnorm", bufs=2)
                nc.sync.dma_start(out=w_sb,
                                  in_=wdram.rearrange("(c p) j -> p c j", p=P))
                sps = ps.tile([P, NDC, NB], f32, tag="stat", bufs=1)
                for jc in range(NDC):
                    for c in range(NCF):
                        mm(sps[:, jc, :], w_sb[:, c, jc * P:(jc + 1) * P],
                           cond_sb[:, c, :], start=(c == 0), stop=(c == NCF - 1))
                nc.vector.tensor_scalar_add(dst[:], sps[:], 1.0)

        # ---- stage B: kT8 (cosine-normalized fp8) and vtE/vtO per batch ----
        kt8_sb = []   # [128(j), NJC, LP] fp8
        ktz_sb = []   # [128(j), NJC, 128] fp8 — c1 keys split per head half,
                      # zero elsewhere, so one M=128 matmul scores both heads
        vtE_sb = []   # [128(l), 2(pair), NH, 64] fp8 — even heads, odd cols 0
        vtO_sb = []   # same — odd heads, even cols 0
        for b in range(NB):
            kt8_sb.append(acts.tile([P, NJC, LP], f8, name=f"kt8{b}"))
            ktz_sb.append(acts.tile([P, NJC, P], f8, name=f"ktz{b}"))
            vtE_sb.append(acts.tile([P, 2, NH, D_HEAD], f8, name=f"vtE{b}"))
            vtO_sb.append(acts.tile([P, 2, NH, D_HEAD], f8, name=f"vtO{b}"))
        with tc.tile_pool(name="pkv", bufs=1) as pkv:
            def b_prep(b):
                """cc load, gamma stats, fp8 quantize of scaled cc."""
                nc.vector.memset(vtE_sb[b][:], 0.0)
                nc.vector.memset(vtO_sb[b][:], 0.0)
                nc.vector.memset(ktz_sb[b][:], 0.0)
                cc = pkv.tile([P, NDC, LP], f16, tag="cc", bufs=2)
                nc.sync.dma_start(out=cc,
                                  in_=ccT[b].rearrange("(c p) l -> p c l", p=P))
                ccsq = pkv.tile([P, NDC, LP], f16, tag="ccsq", bufs=1)
                nc.gpsimd.tensor_mul(ccsq[:], cc[:], cc[:])
                msq = ps.tile([1, LP], f32, tag="stat", bufs=1)
                for c in range(NDC):
                    mm(msq, ones, ccsq[:, c, :], start=(c == 0), stop=(c == NDC - 1))
                gr = small.tile([1, LP], f32, tag="gamr")
                act_raw(gr, msq, Rsqrt, eps256[:1], WS * WS / DC)
                grpad = small.tile([1, P], f32, tag="grpad")
                nc.vector.memset(grpad[:, 0:D_HEAD], 0.0)
                nc.vector.tensor_copy(out=grpad[:, D_HEAD:P], in_=gr[:, P:LP])
                gps = ps.tile([P, 3], f32, tag="stat", bufs=1)
                nc.tensor.matmul(gps[:, 0:1], gr[0:1, 0:P], onef[:],
                                 start=True, stop=True)
                nc.tensor.matmul(gps[0:D_HEAD, 1:2], gr[0:1, P:LP], onef[:],
                                 start=True, stop=True)
                nc.tensor.matmul(gps[:, 2:3], grpad[0:1, :], onef[:],
                                 start=True, stop=True)
                nc.scalar.copy(out=gam[:, :, b], in_=gps)
                cc8 = pkv.tile([P, NDC, 320], f8, tag="cc8", bufs=2)
                nc.vector.memset(cc8[:, :, LP:256], 0.0)
                for c in range(NDC):
                    nc.vector.tensor_scalar_mul(cc8[:, c, 0:LP], cc[:, c, :],
                                                s_c[:, c, b:b + 1])
                    nc.vector.tensor_scalar_mul(cc8[:, c, 256:320],
                                                cc[:, c, P:LP],
                                                s_c[:, c, b:b + 1])
                return cc8

            def b_kproj(b, cc8):
                """k projection + norm stats."""
                ktf = pkv.tile([P, NJC, LP], f16, tag="ktf", bufs=2)
                ksq = pkv.tile([P, NJC, LP], f16, tag="ksq", bufs=1)
                kss = ps.tile([NH, LP], f32, tag="stat", bufs=1)
                for jc in range(NJC):
                    kps = ps.tile([P, CH], f32, tag="qps", bufs=2)
                    for i in range(NKP):
                        mm8(kps[:, 0:LP],
                            wkv8[:, 2 * i:2 * i + 2, jc * P:(jc + 1) * P],
                            cc8[:, 2 * i:2 * i + 2, 0:LP],
                            start=(i == 0), stop=(i == NKP - 1))
                    nc.scalar.copy(out=ktf[:, jc, :], in_=kps[:, 0:LP])
                    nc.gpsimd.tensor_mul(ksq[:, jc, :], ktf[:, jc, :],
                                         ktf[:, jc, :])
                    mm(kss, indT_sb[:, jc, :], ksq[:, jc, :],
                       start=(jc == 0), stop=(jc == NJC - 1))
                gkT = small.tile([NH, LP], f16, tag="gkT")
                act_raw(gkT, kss, Rsqrt, scl[:, 3:4], scl[:, 2:3])
                gkd = dscr.tile([NH, LP], f16, tag="gkd", bufs=2)
                nc.sync.dma_start(out=gkd, in_=gkT)
                gkbs = []
                for jc in range(NJC):
                    gkb = pkv.tile([P, LP], f16, tag="gkb", bufs=2 * NJC)
                    nc.sync.dma_start(
                        out=gkb[0:D_HEAD, :],
                        in_=gkd[2 * jc:2 * jc + 1, :].to_broadcast((D_HEAD, LP)))
                    nc.sync.dma_start(
                        out=gkb[D_HEAD:P, :],
                        in_=gkd[2 * jc + 1:2 * jc + 2, :].to_broadcast((D_HEAD, LP)))
                    gkbs.append(gkb)
                return ktf, gkbs

            def b_vproj(b, cc8):
                """v projection, gamma scale, parity split into vtE/vtO."""
                vtE, vtO = vtE_sb[b], vtO_sb[b]
                for vjc in range(2):
                    vps = ps.tile([P, CH], f32, tag="att", bufs=4)
                    for i in range(NKP):
                        mm8(vps, cc8[:, 2 * i:2 * i + 2, 0:P],
                            wkv8[:, 2 * i:2 * i + 2, D + vjc * CH:D + (vjc + 1) * CH],
                            start=(i == 0), stop=(i == NKP - 1))
                    he = vps.rearrange("p (h e) -> p h e", e=D_HEAD)
                    nc.vector.tensor_scalar_mul(
                        vtE[:, 0, vjc * 8:(vjc + 1) * 8:2, :],
                        he[:, 0::2, :], gam[:, 0, b:b + 1])
                    nc.vector.tensor_scalar_mul(
                        vtO[:, 1, vjc * 8 + 1:(vjc + 1) * 8:2, :],
                        he[:, 1::2, :], gam[:, 0, b:b + 1])
                    vpsA = ps.tile([P, CH], f32, tag="att", bufs=4)
                    for i in range(NKP):
                        mm8(vpsA[0:D_HEAD, :], cc8[:, 2 * i:2 * i + 2, P:LP],
                            wkv8[:, 2 * i:2 * i + 2, D + vjc * CH:D + (vjc + 1) * CH],
                            start=(i == 0), stop=(i == NKP - 1))
                    heA = vpsA.rearrange("p (h e) -> p h e", e=D_HEAD)
                    nc.vector.tensor_scalar_mul(
                        vtE[0:D_HEAD, 1, vjc * 8:(vjc + 1) * 8:2, :],
                        heA[0:D_HEAD, 0::2, :], gam[0:D_HEAD, 1, b:b + 1])
                    vpsB = ps.tile([P, CH], f32, tag="att", bufs=4)
                    for i in range(NKP):
                        mm8(vpsB, cc8[:, 2 * i:2 * i + 2, 192:320],
                            wkv8[:, 2 * i:2 * i + 2, D + vjc * CH:D + (vjc + 1) * CH],
                            start=(i == 0), stop=(i == NKP - 1))
                    heB = vpsB.rearrange("p (h e) -> p h e", e=D_HEAD)
                    nc.vector.tensor_scalar_mul(
                        vtO[D_HEAD:P, 0, vjc * 8 + 1:(vjc + 1) * 8:2, :],
                        heB[D_HEAD:P, 1::2, :], gam[D_HEAD:P, 2, b:b + 1])

            def b_knorm(b, ktf, gkbs):
                kt8, ktz = kt8_sb[b], ktz_sb[b]
                for jc in range(NJC):
                    nc.vector.tensor_tensor(kt8[:, jc, 0:P], ktf[:, jc, 0:P],
                                            gkbs[jc][:, 0:P], MULT)
                    nc.vector.tensor_tensor(
                        ktz[0:D_HEAD, jc, 0:D_HEAD],
                        ktf[0:D_HEAD, jc, P:LP],
                        gkbs[jc][0:D_HEAD, P:LP], MULT)
                    nc.vector.tensor_tensor(
                        ktz[D_HEAD:P, jc, D_HEAD:P],
                        ktf[D_HEAD:P, jc, P:LP],
                        gkbs[jc][D_HEAD:P, P:LP], MULT)

            stageB = {"prep": b_prep, "kproj": b_kproj, "vproj": b_vproj,
                      "knorm": b_knorm}

            # ---- stages C/D/E: stream 512-token chunks ----
            pw2 = pkv
            NCH = NB * (T // CH)

            def phase1(chunk):
                """xt load + xq8 quantize only."""
                b, th = chunk // (T // CH), chunk % (T // CH)
                tsl = slice(th * CH, (th + 1) * CH)
                xt = pw2.tile([P, NDC, CH], f16, tag="xt", bufs=2)
                nc.sync.dma_start(
                    out=xt, in_=xT[b].rearrange("(c p) t -> p c t", p=P)[:, :, tsl])
                xq8 = pw2.tile([P, NDC, CH], f8, tag="xq8", bufs=2)
                for c in range(NDC):
                    nc.vector.tensor_scalar_mul(xq8[:, c, :], xt[:, c, :],
                                                s_x[:, c, b:b + 1])
                return {"b": b, "th": th, "xq8": xq8}

            def qproj_work(st1):
                """Returns (8 per-jc filler closures, finalize) for the q proj
                of the chunk described by st1."""
                b, xq8 = st1["b"], st1["xq8"]
                qf = pw2.tile([P, NJC, CH], f16, tag="qf", bufs=2)
                qsq = pw2.tile([P, NJC, CH], f16, tag="qsq", bufs=1)
                qss = ps.tile([NH, CH], f32, tag="stat", bufs=1)

                def filler(jc):
                    qps = ps.tile([P, CH], f32, tag="qps", bufs=2)
                    for i in range(NKP):
                        mm8(qps, wq8[:, 2 * i:2 * i + 2, jc * P:(jc + 1) * P],
                            xq8[:, 2 * i:2 * i + 2, :],
                            start=(i == 0), stop=(i == NKP - 1))
                    nc.scalar.copy(out=qf[:, jc, :], in_=qps)
                    nc.gpsimd.tensor_mul(qsq[:, jc, :], qf[:, jc, :],
                                         qf[:, jc, :])
                    mm(qss, indT_sb[:, jc, :], qsq[:, jc, :],
                       start=(jc == 0), stop=(jc == NJC - 1))

                def finalize():
                    gqT = small.tile([NH, CH], f16, tag="gqT")
                    act_raw(gqT, qss, Rsqrt, scl[:, 1:2], scl[:, 0:1])
                    gqd = dscr.tile([NH, CH], f16, tag="gqd", bufs=2)
                    nc.sync.dma_start(out=gqd, in_=gqT)
                    gqbs = []
                    for m in range(NJC):
                        gqb = pw2.tile([P, CH], f16, tag="gqb", bufs=12)
                        nc.sync.dma_start(
                            out=gqb[0:D_HEAD, :],
                            in_=gqd[2 * m:2 * m + 1, :].to_broadcast((D_HEAD, CH)))
                        nc.sync.dma_start(
                            out=gqb[D_HEAD:P, :],
                            in_=gqd[2 * m + 1:2 * m + 2, :].to_broadcast(
                                (D_HEAD, CH)))
                        gqbs.append(gqb)
                    return {"b": st1["b"], "th": st1["th"], "qf": qf,
                            "gqbs": gqbs}

                return [lambda jc=jc: filler(jc) for jc in range(NJC)], finalize

            def phase2(st, nxt1=None):
                """Attention pair loop; embeds the next chunk's q proj as
                PE bubble-filler, plus its xq8 quantize ops spread early."""
                b, qf, gqbs = st["b"], st["qf"], st["gqbs"]
                kt8, vtE, vtO = kt8_sb[b], vtE_sb[b], vtO_sb[b]
                ktz = ktz_sb[b]
                fillers, finalize = (qproj_work(nxt1) if nxt1 is not None
                                     else (None, None))
                q8 = pw2.tile([P, NJC, CH], f8, tag="q8", bufs=2)
                of = pw2.tile([P, NJC, CH], f16, tag="of", bufs=1)
                dends = []
                for m in range(NJC):
                    h0, h1 = 2 * m, 2 * m + 1
                    nc.vector.tensor_tensor(q8[:, m, :], qf[:, m, :], gqbs[m],
                                             MULT)
                    E2 = pw2.tile([P, 3, CH], f8, tag="E2", bufs=4)
                    scpA = ps.tile([P, CH], f32, tag="att", bufs=4)
                    mm(scpA, kt8[0:D_HEAD, m, 0:P], q8[0:D_HEAD, m, :],
                       start=True, stop=True)
                    nc.scalar.activation(out=E2[:, 0, :], in_=scpA, func=Exp,
                                         bias=mb_sb[:, 0, b:b + 1], scale=1.0)
                    scpB = ps.tile([P, CH], f32, tag="att", bufs=4)
                    mm(scpB, ktz[:, m, :], q8[:, m, :], start=True, stop=True)
                    nc.scalar.activation(out=E2[:, 1, :], in_=scpB, func=Exp,
                                         bias=mb_sb[:, 1, b:b + 1], scale=1.0)
                    scpC = ps.tile([P, CH], f32, tag="att", bufs=4)
                    mm(scpC, kt8[D_HEAD:P, m, 0:P], q8[D_HEAD:P, m, :],
                       start=True, stop=True)
                    nc.scalar.activation(out=E2[:, 2, :], in_=scpC, func=Exp,
                                         bias=mb_sb[:, 0, b:b + 1], scale=1.0)
                    if fillers is not None and 3 <= m:
                        fillers[m - 3]()
                    den_m = ps.tile([2, CH], f32, tag="den", bufs=1)
                    mm8(den_m, dpa[:, :, h0:h0 + 2], E2[:, 0:2, :],
                        start=True, stop=False)
                    mm8(den_m, dpb[:, :, h0:h0 + 2], E2[:, 1:3, :],
                        start=False, stop=True)
                    oap = ps.tile([P, CH], f32, tag="att", bufs=4)
                    mm8(oap, vtE[:, :, h0:h0 + 2, :], E2[:, 0:2, :],
                        start=True, stop=False)
                    mm8(oap, vtO[:, :, h0:h0 + 2, :], E2[:, 1:3, :],
                        start=False, stop=True)
                    nc.vector.tensor_copy(out=of[:, m, :], in_=oap)
                    den_f = small.tile([2, CH], f32, tag="denf")
                    nc.scalar.copy(out=den_f, in_=den_m)
                    den_r = small.tile([2, CH], f32, tag="denr")
                    nc.vector.reciprocal_approx_fast(out=den_r, in_=den_f)
                    dend = dscr.tile([2, CH], f32, tag="dend", bufs=4)
                    nc.sync.dma_start(out=dend, in_=den_r)
                    dends.append(dend)
                nxt = None
                if fillers is not None:
                    for jc in range(NJC - 3, NJC):
                        fillers[jc]()
                    nxt = finalize()
                st.update({"of": of, "dends": dends})
                return st, nxt

            def phase3(st):
                """Divide by denominators, out projection, skip add, store."""
                b, th, of, dends = st["b"], st["th"], st["of"], st["dends"]
                o8 = pw2.tile([P, NJC, CH], f8, tag="o8", bufs=2)
                dbps = []
                for m in range(NJC):
                    dbp = pw2.tile([P, CH], f32, tag="dbpb", bufs=NJC)
                    nc.sync.dma_start(
                        out=dbp[0:D_HEAD, :],
                        in_=dends[m][0:1, :].to_broadcast((D_HEAD, CH)))
                    nc.sync.dma_start(
                        out=dbp[D_HEAD:P, :],
                        in_=dends[m][1:2, :].to_broadcast((D_HEAD, CH)))
                    dbps.append(dbp)
                for m in range(NJC):
                    nc.vector.tensor_tensor(o8[:, m, :], of[:, m, :], dbps[m],
                                            MULT)
                for t4 in range(CH // P):
                    trow = th * CH + t4 * P
                    xs = small.tile([P, D], f16, tag="xs")
                    nc.sync.dma_start(out=xs, in_=xs_d[b, trow:trow + P, :])
                    os_ = small.tile([P, D], f16, tag="os")
                    for d2 in range(2):
                        ops = ps.tile([P, CH], f32, tag="qps", bufs=2)
                        for i in range(NKP):
                            mm8(ops, o8[:, 2 * i:2 * i + 2, t4 * P:(t4 + 1) * P],
                                wo8[:, 2 * i:2 * i + 2, d2 * CH:(d2 + 1) * CH],
                                start=(i == 0), stop=(i == NKP - 1))
                        nc.vector.tensor_tensor(os_[:, d2 * CH:(d2 + 1) * CH], ops,
                                                xs[:, d2 * CH:(d2 + 1) * CH], ADD)
                    nc.sync.dma_start(out=out[b, trow:trow + P, :], in_=os_)

            # schedule: stage B b=0 first, chunk0 qproj standalone, then the
            # pipelined megas with stage B b=1 slotted between early chunks.
            cc80 = stageB["prep"](0)
            st1_0 = phase1(0)
            ks0 = stageB["kproj"](0, cc80)
            stageB["vproj"](0, cc80)
            stageB["knorm"](0, *ks0)
            f0, fin0 = qproj_work(st1_0)
            for f in f0:
                f()
            st0 = fin0()
            cc81 = stageB["prep"](1)
            st1_1 = phase1(1)
            st2, stA = phase2(st0, st1_1)
            ks1 = stageB["kproj"](1, cc81)
            st1_2 = phase1(2)
            phase3(st2)
            stageB["vproj"](1, cc81)
            stageB["knorm"](1, *ks1)
            st2, stB = phase2(stA, st1_2)
            st1_3 = phase1(3)
            phase3(st2)
            st2, stC = phase2(stB, st1_3)
            phase3(st2)
            st2, _ = phase2(stC, None)
            phase3(st2)

    nc.compile()
    return nc


def _prep_inputs(x, cond, crossattn_cond, crossattn_mask, w_norm, w_q, w_cnorm,
                 w_kv, qk_scale, w_o):
    """Shard + lay out the full inputs into 8 per-core input maps."""
    f = np.float32
    h = np.float16
    e4 = ml_dtypes.float8_e4m3

    # den ones pattern: col h nonzero = WS; even h: (slot0 all, slot1 rows 0:64);
    # odd h: (slot0 rows 64:128, slot1 all)
    dpa = np.zeros((128, 2, NH), e4)
    dpb = np.zeros((128, 2, NH), e4)
    for hh in range(NH):
        if hh % 2 == 0:
            dpa[:, 0, hh] = e4(WS)
            dpa[0:64, 1, hh] = e4(WS)
        else:
            dpb[64:128, 0, hh] = e4(WS)
            dpb[:, 1, hh] = e4(WS)
    shared = {
        "dpa": dpa,
        "dpb": dpb,
        "w_nT": np.ascontiguousarray(w_norm.T).astype(h),
        "w_cT": np.ascontiguousarray(w_cnorm.T).astype(h),
        "wq8": (np.ascontiguousarray(w_q.T) * WS).astype(e4),
        "wkv8": (np.ascontiguousarray(w_kv.T) * WS).astype(e4),
        "wo8": (np.ascontiguousarray(w_o.T) * WS).astype(e4),
        "indT": np.kron(np.eye(NH, dtype=h), np.ones((D_HEAD, 1), dtype=h)),
        "scl": np.stack([64.0 / qk_scale.astype(f),
                         EPS * 64.0 / qk_scale.astype(f),
                         1.0 / qk_scale.astype(f),
                         EPS / qk_scale.astype(f)], axis=1).astype(f),
        "onesd": np.ones((128, 1), dtype=h),
        "onesf": np.ones((1, 1), dtype=f),
    }
    in_maps = []
    for c in range(NCORES):
        s = slice(c * NB, (c + 1) * NB)
        xc = np.ascontiguousarray(x[s], dtype=f).reshape(NB, T, D)
        ccg = np.zeros((NB, DC, LP), h)
        mb = np.full((NB, 2, 128), f(MASK_NEG), f)
        for b in range(NB):
            idx = np.nonzero(crossattn_mask[s][b])[0]
            cnt = len(idx)
            assert cnt <= LP, f"mask count {cnt} exceeds LP={LP}"
            ccg[b, :, :cnt] = crossattn_cond[s][b][idx].T.astype(h)
            mb[b, 0, :min(cnt, 128)] = 0.0
            if cnt > 128:
                r = cnt - 128
                mb[b, 1, 0:r] = 0.0
                mb[b, 1, 64:64 + r] = 0.0
        m = {
            "xs": xc.astype(h),
            "xT": np.ascontiguousarray(xc.transpose(0, 2, 1)).astype(h),
            "ccT": ccg,
            "condT": np.ascontiguousarray(cond[s].T, dtype=f).astype(h),
            "mb": mb,
        }
        m.update(shared)
        in_maps.append(m)
    return in_maps


def _run(inputs, trace=False):
    from concourse.bass_utils import run_bass_kernel_spmd

    if "nc" not in _cached:
        _cached["nc"] = _build_nc()
    nc = _cached["nc"]
    in_maps = _prep_inputs(**inputs)
    res = run_bass_kernel_spmd(nc, in_maps, core_ids=list(range(NCORES)),
                               trace=trace)
    outs = np.concatenate([r["out"] for r in res.results], axis=0)
    return outs.reshape(N, H, W, D).astype(np.float32), res


def kernel(**inputs):
    out, _ = _run(inputs, trace=False)
    return out
